# revision 22
# baseline (speedup 1.0000x reference)
"""Trainium2 Bass kernel for a 3D non-local attention block.

Math (per batch b):
  xf = x.reshape(C, N)                         C=64, N=32768 (=32^3)
  theta = w_theta @ xf                         [8, N]
  phi   = maxpool2(w_phi @ xf)                 [8, M], M=4096
  g     = maxpool2(w_g   @ xf)                 [32, M]
  beta  = softmax_over_m(theta^T phi)          [N, M]
  o     = g @ beta^T                           [32, N]
  out   = gamma * (w_o @ o) + xf               [C, N]

Sharding: 8 cores, core k -> batch k//4, query slice k%4 (8192 queries).
Every core re-computes the (cheap) pooled phi/g from the full batch and
runs flash-style attention over its own query slice; no collectives.

v2: the exp(S) stream (33.5M elems/core, the hard floor at ~1 elem/
cycle/lane on ScalarE) is split across TWO engines: ScalarE runs table
exp on ~2/3 of the m-chunk groups and the DVE runs a Schraudolph
bit-trick exp (int16(A*x+B) bitcast to bf16) on the rest.  Pooling is
restructured so projection matmuls col-tile two 512-col x-chunks into
one [128,512] PSUM bank and the 2x2x2 maxpool runs as strided TT max
(fp32 PSUM) then 2x-mode bf16 TTs.  G' is built with DMA transposes (no
PE/ScalarE cost).  The o-matmuls accumulate even/odd chunks into ONE
PSUM bank (partitions 0:64 / 64:128) and the softmax denominator merge
folds into a single [33,F] TT.  PSUM: psS 2x3 banks + psO 1 + psA 1 = 8.
"""

import os
import sys

sys.path.insert(0, "/opt/trn_rl_repo")

import numpy as np

C = 64            # channels
N = 32768         # voxels (32^3)
NS = N // 4       # query slice per core (8192)
M = N // 8        # pooled keys (4096)
F = 512           # free-dim tile (PSUM bank)
NT = NS // F      # 16 n-tiles per core
MC = M // 128     # 32 m-chunks of 128
GROUPS = [(s, min(s + 3, MC)) for s in range(0, MC, 3)]  # 11 groups (last=2)
NG = len(GROUPS)

# Schraudolph constants: exp(x) ~ bitcast_bf16(int16(A16*x + B16))
A16 = float(2.0**7 / np.log(2.0))
B16 = float(127.0 * 2.0**7 - 366393.0 / 65536.0)

# Which exp groups go to the DVE (Schraudolph) per tile. Tiles 0-1 are
# all-ScalarE so the DVE can finish the phase-A pooling backlog.
DVE_GROUPS_EVEN = (2, 5, 8)
DVE_GROUPS_ODD = (2, 6, 9)


def _dve_groups(t):
    if os.environ.get("NLATTN_NO_DVE_EXP"):
        return ()
    if t < 2:
        return (5, 9)
    return DVE_GROUPS_EVEN if t % 2 == 0 else DVE_GROUPS_ODD


def _build_program():
    import concourse.bass as bass  # noqa: F401
    import concourse.tile as tile
    from concourse import bacc, mybir

    f32 = mybir.dt.float32
    f32r = mybir.dt.float32r
    bf16 = mybir.dt.bfloat16
    fp16 = mybir.dt.float16
    i16 = mybir.dt.int16

    nc = bacc.Bacc()

    x_full = nc.declare_dram_parameter("x_full", [C, N], f32, isOutput=False)
    x_slice = nc.declare_dram_parameter("x_slice", [C, NS], f32, isOutput=False)
    w_pg = nc.declare_dram_parameter("w_pg", [C, 64], f32, isOutput=False)
    w_th = nc.declare_dram_parameter("w_th", [C, 32], f32, isOutput=False)
    w_oT = nc.declare_dram_parameter("w_oT", [32, C], f32, isOutput=False)
    gamma = nc.declare_dram_parameter("gamma", [1, 1], f32, isOutput=False)
    out_d = nc.declare_dram_parameter("out", [C, NS], f32, isOutput=True)
    DBG = bool(os.environ.get("NLATTN_DEBUG"))
    if DBG:
        dbg_phi = nc.declare_dram_parameter("dbg_phi", [72, M], f32, isOutput=True)
        dbg_th = nc.declare_dram_parameter("dbg_th", [96, NS], f32, isOutput=True)
        dbg_gt = nc.declare_dram_parameter("dbg_gt", [128, MC * 64], f32, isOutput=True)
        dbg_o33 = nc.declare_dram_parameter("dbg_o33", [33, F], f32, isOutput=True)
        dbg_den = nc.declare_dram_parameter("dbg_den", [1, F], f32, isOutput=True)
        dbg_exp = nc.declare_dram_parameter("dbg_exp", [128, MC, F], f32, isOutput=True)

    Exp = mybir.ActivationFunctionType.Exp
    Max = mybir.AluOpType.max
    Add = mybir.AluOpType.add
    Mult = mybir.AluOpType.mult

    with tile.TileContext(nc) as tc:
        with (
            tc.tile_pool(name="consts", bufs=1) as consts,
            tc.tile_pool(name="big", bufs=2) as bigpool,
            tc.tile_pool(name="s1p", bufs=2) as s1pool,
            tc.tile_pool(name="s2p", bufs=2) as s2pool,
            tc.tile_pool(name="gpp", bufs=2) as gppool,
            tc.tile_pool(name="theta", bufs=1) as thpool,
            tc.tile_pool(name="pg", bufs=1) as pgpool,
            tc.tile_pool(name="gtp", bufs=1) as gtpool,
            tc.tile_pool(name="th4p", bufs=2) as th4pool,
            tc.tile_pool(name="xin", bufs=3) as xpool,
            tc.tile_pool(name="small", bufs=2) as smallpool,
            tc.tile_pool(name="outp", bufs=2) as outpool,
        ):
            psS = tc.alloc_tile_pool(name="psS", bufs=2, space="PSUM")
            psO_p = tc.alloc_tile_pool(name="psO", bufs=1, space="PSUM")
            psA = tc.alloc_tile_pool(name="psA", bufs=1, space="PSUM")
            w_pg_sb = consts.tile([C, 64], fp16)
            nc.gpsimd.dma_start(out=w_pg_sb, in_=w_pg[:])
            w_th_sb = consts.tile([C, 32], fp16)
            nc.gpsimd.dma_start(out=w_th_sb, in_=w_th[:])
            gamma_sb = consts.tile([1, 1], f32)
            nc.sync.dma_start(out=gamma_sb, in_=gamma[:])
            w_oT_f32 = consts.tile([32, C], f32)
            nc.sync.dma_start(out=w_oT_f32, in_=w_oT[:])
            w_oT_sb = consts.tile([32, C], f32r)
            g32 = consts.tile([32, 1], f32)
            nc.gpsimd.partition_broadcast(g32, gamma_sb)
            nc.vector.tensor_scalar_mul(w_oT_sb, w_oT_f32, g32)
            ones32 = consts.tile([128, 32], f32)
            nc.vector.memset(ones32, 1.0)
            zeros_sb = consts.tile([128, F], f32)
            nc.vector.memset(zeros_sb, 0.0)

            # HAM warm-up: ~6us of dense back-to-back matmuls flips the PE
            # clock gate from 4/8 (1.2 GHz) to 8/8 (2.4 GHz); the steady-state
            # pipeline never idles the PE >3.4us, so it stays warm after.
            prim_w = consts.tile([64, 64], bf16)
            nc.vector.memset(prim_w, 0.0)
            prim_x = consts.tile([64, F], bf16)
            nc.vector.memset(prim_x, 0.0)
            prim_ps = psO_p.tile([128, F], f32, name="prim_ps", tag="psO")
            for _ in range(14):
                nc.tensor.matmul(
                    prim_ps[0:64, :], prim_w, prim_x, start=True, stop=True
                )

            # phi replicas at partition offsets 0/32/64 for row-tiled S.
            phi_sb = pgpool.tile([72, M], bf16)
            theta_sb = thpool.tile([96, NS], bf16, tag="th96")

            # G' = [g^T | 1], zero-padded to 64 columns, chunk-major.
            gt = gtpool.tile([128, MC, 64], bf16)
            gtv = gt.rearrange("p a b -> p (a b)")
            for z0 in range(0, MC * 64, F):
                nc.scalar.copy(gtv[:, z0 : z0 + F], zeros_sb[:, 0:F])
            nc.scalar.copy(gt[:, :, 32], ones32)

            # ---------------- PSUM pools (shared by phase A and C) -----------
            def phase_a_quarter_steps(q):
                """Emission callbacks for quarter q's projection + 2x2x2
                maxpool; lazily allocates its tiles at first call."""
                st = {}
                m0 = q * 1024

                def fill(p):
                    # one [128, F] psA bank <- 1024 x-cols (one d-slice piece,
                    # h-halves on the partition axis)
                    def go():
                        if p == 0:
                            st["s1"] = s1pool.tile([128, 2048], bf16, tag="s1", name="s1")
                            st["s2"] = s2pool.tile([128, 1024], bf16, tag="s2", name="s2")
                            st["gp"] = gppool.tile([128, 512], bf16, tag="gp", name="gp")
                        if p % 2 == 0:
                            xc = xpool.tile([C, 2048], fp16, tag="x")
                            base = q * 8192 + (p // 2) * 2048
                            nc.gpsimd.dma_start(out=xc, in_=x_full[:, base : base + 2048])
                            st["xc"] = xc
                        xc = st["xc"]
                        u = (p % 2) * 1024
                        ps = psA.tile([128, F], f32, tag="pgbank", name="psAfill")
                        nc.tensor.matmul(
                            ps[0:64, :], w_pg_sb, xc[:, u : u + 512],
                            start=True, stop=True, tile_position=(0, 0),
                        )
                        nc.tensor.matmul(
                            ps[64:128, :], w_pg_sb, xc[:, u + 512 : u + 1024],
                            start=True, stop=True, tile_position=(0, 64),
                        )
                        nc.vector.tensor_reduce(
                            st["s1"][:, p * 256 : (p + 1) * 256],
                            ps.rearrange("c (a two) -> c a two", two=2),
                            mybir.AxisListType.X,
                            Max,
                        )
                    return go

                for p in range(8):
                    yield fill(p)

                def hpool():
                    v = st["s1"].rearrange(
                        "c (p j two w) -> c p j two w", p=8, j=8, two=2, w=16
                    )
                    nc.vector.tensor_tensor(
                        st["s2"], v[:, :, :, 0, :], v[:, :, :, 1, :], Max
                    )
                yield hpool

                def dpool():
                    v = st["s2"].rearrange(
                        "c (pp two r) -> c pp two r", pp=4, two=2, r=128
                    )
                    nc.vector.tensor_tensor(st["gp"], v[:, :, 0, :], v[:, :, 1, :], Max)
                yield dpool

                def shuffle():
                    gp = st["gp"]
                    # phi rows (32:40 / 96:104 of gp) -> phi_sb replicas.
                    # gp free=(pp,j,w): local m = pp*256 + hhalf*128 + j*16 + w
                    for hh in range(2):
                        src = gp[hh * 64 + 32 : hh * 64 + 40, :].rearrange(
                            "c (pp r) -> c pp r", pp=4
                        )
                        for off in (0, 32, 64):
                            dst = phi_sb[off : off + 8, m0 : m0 + 1024].rearrange(
                                "c (pp two r) -> c pp two r", pp=4, two=2
                            )
                            nc.gpsimd.dma_start(out=dst[:, :, hh, :], in_=src)
                    # G' chunks via DMA transpose: chunk (q*8 + pp*2 + hh)
                    for cl in range(8):
                        pp, hh = cl // 2, cl % 2
                        nc.sync.dma_start(
                            out=gt[:, q * 8 + cl, 0:32],
                            in_=gp[hh * 64 : hh * 64 + 32, pp * 128 : (pp + 1) * 128],
                            transpose=True,
                        )
                yield shuffle

            def theta_steps():
                """theta projection from x_slice: 4 bank-fills, each 4-way
                col-tiled (2048 slice-cols per bank), ScalarE evacuation."""
                def fill(tf):
                    def go():
                        xt = xpool.tile([C, 2048], fp16, tag="x")
                        nc.gpsimd.dma_start(
                            out=xt, in_=x_slice[:, tf * 2048 : (tf + 1) * 2048]
                        )
                        th = psA.tile([128, F], f32, tag="pgbank", name="psThfill")
                        for c in range(4):
                            nc.tensor.matmul(
                                th[32 * c : 32 * c + 32, :],
                                w_th_sb,
                                xt[:, c * 512 : (c + 1) * 512],
                                start=True, stop=True, tile_position=(0, 32 * c),
                            )
                        th4 = th4pool.tile([128, F], bf16, tag="th4")
                        nc.scalar.copy(th4, th)
                        for c in range(4):
                            nc.gpsimd.dma_start(
                                out=theta_sb[
                                    0:8, tf * 2048 + c * 512 : tf * 2048 + (c + 1) * 512
                                ],
                                in_=th4[32 * c : 32 * c + 8, :],
                            )
                    return go

                for tf in range(4):
                    yield fill(tf)

                def replicate():
                    for off in (32, 64):
                        nc.gpsimd.dma_start(
                            out=theta_sb[off : off + 8, :], in_=theta_sb[0:8, :]
                        )
                yield replicate

            a_steps = list(theta_steps())
            for q in range(4):
                a_steps.extend(phase_a_quarter_steps(q))
            a_idx = 0

            def drain_a(k):
                nonlocal a_idx
                for _ in range(k):
                    if a_idx < len(a_steps):
                        a_steps[a_idx]()
                        a_idx += 1

            # Emit theta + quarter 0 up front; quarters 1-3 interleave below.
            drain_a(5 + 11)

            # ---------------- Phase C: flash attention -----------------------
            if True:
                def make_state(t):
                    return {
                        "t": t,
                        "n0": t * F,
                        "expS": bigpool.tile([128, MC, F], bf16, tag="big", name="expS"),
                        "psO": psO_p.tile([128, F], f32, name="psO", tag="psO"),
                        "ready": 0,
                        "odone": 0,
                        "ne": 0,
                        "no": 0,
                    }

                def emit_group(st, gi):
                    mc0, mc1 = GROUPS[gi]
                    cnt = mc1 - mc0
                    sps = psS.tile([128, 3 * F], f32, tag="psS", name="sps")
                    for i, mc in enumerate(range(mc0, mc1)):
                        nc.tensor.matmul(
                            sps[:, i * F : (i + 1) * F],
                            phi_sb[32 * i : 32 * i + 8, mc * 128 : (mc + 1) * 128],
                            theta_sb[32 * i : 32 * i + 8, st["n0"] : st["n0"] + F],
                            start=True,
                            stop=True,
                            tile_position=(32 * i, 0),
                        )
                    if gi in _dve_groups(st["t"]):
                        nc.vector.tensor_scalar(
                            st["expS"][:, mc0:mc1, :].bitcast(i16),
                            sps[:, 0 : cnt * F],
                            A16,
                            B16,
                            Mult,
                            Add,
                        )
                    else:
                        nc.scalar.activation(
                            out=st["expS"][:, mc0:mc1, :],
                            in_=sps[:, 0 : cnt * F],
                            func=Exp,
                        )
                    st["ready"] = mc1

                def emit_o(st):
                    mc = st["odone"]
                    par = mc % 2
                    first = (st["ne"] if par == 0 else st["no"]) == 0
                    last = (st["ne"] if par == 0 else st["no"]) == MC // 2 - 1
                    nc.tensor.matmul(
                        st["psO"][0:64, :] if par == 0 else st["psO"][64:128, :],
                        gt[:, mc, :],
                        st["expS"][:, mc, :],
                        start=first,
                        stop=last,
                        tile_position=(0, 0) if par == 0 else (0, 64),
                    )
                    if par == 0:
                        st["ne"] += 1
                    else:
                        st["no"] += 1
                    st["odone"] += 1

                def tail_part1(st):
                    # merge even/odd sums; separate den TT (the custom-DVE
                    # reciprocal requires a base-0 in-place operand)
                    psO = st["psO"]
                    tmp = smallpool.tile([33, F], f32, tag="ob")
                    nc.vector.tensor_copy(tmp, psO[64:97, :])
                    o32 = smallpool.tile([32, F], f32r, tag="o33")
                    nc.vector.tensor_tensor(o32, psO[0:32, :], tmp[0:32, :], Add)
                    den = smallpool.tile([1, F], f32, tag="den")
                    nc.vector.tensor_tensor(den, psO[32:33, :], tmp[32:33, :], Add)
                    st["o32"] = o32
                    st["den"] = den
                    if DBG and st["t"] == 0:
                        d1 = outpool.tile([33, F], f32, name="d1", tag="dbg1", bufs=1)
                        nc.vector.tensor_copy(d1[0:32], o32[:].bitcast(f32))
                        nc.vector.tensor_copy(d1[32:33], den)
                        nc.sync.dma_start(out=dbg_o33[:], in_=d1)
                        for pc in range(4):
                            d2 = outpool.tile([128, MC // 4, F], f32, name="d2", tag="dbg2", bufs=1)
                            nc.vector.tensor_copy(d2, st["expS"][:, pc * 8 : (pc + 1) * 8, :])
                            nc.sync.dma_start(out=dbg_exp[:, pc * 8 : (pc + 1) * 8, :], in_=d2)
                    xres = xpool.tile([C, F], f32, tag="xres", bufs=2)
                    nc.gpsimd.dma_start(out=xres, in_=x_slice[:, st["n0"] : st["n0"] + F])
                    st["xres"] = xres

                def tail_part2(st):
                    den = st["den"]
                    nc.vector.reciprocal_approx_fast(out=den, in_=den)
                    rb = smallpool.tile([64, F], f32, tag="rb")
                    nc.gpsimd.partition_broadcast(rb, den)
                    st["rb"] = rb
                    if DBG and st["t"] == 0:
                        nc.sync.dma_start(out=dbg_den[:], in_=den)
                    # project unnormalized o; (w_o@o)/den == w_o@(o/den)
                    nc.tensor.matmul(
                        st["psO"][0:64, :],
                        w_oT_sb,
                        st["o32"],
                        start=True,
                        stop=True,
                    )

                def tail_part3(st):
                    pn = smallpool.tile([64, F], f32, tag="pn")
                    nc.vector.tensor_mul(pn, st["psO"][0:64, :], st["rb"])
                    ot = outpool.tile([C, F], f32)
                    nc.vector.tensor_add(ot, pn, st["xres"])
                    nc.gpsimd.dma_start(out=out_d[:, st["n0"] : st["n0"] + F], in_=ot)

                prev = None
                for t in range(NT):
                    st = make_state(t)
                    for gi in range(NG):
                        drain_a(4 if t < 2 else 2)
                        emit_group(st, gi)
                        if prev is not None:
                            # tails early: the single psO bank must be fully
                            # read before this tile's o-matmuls start
                            if gi == 0:
                                tail_part1(prev)
                            elif gi == 1:
                                tail_part2(prev)
                            elif gi == 2:
                                tail_part3(prev)
                                prev = None
                        else:
                            # lag o-matmuls ~2 groups behind exp (sems are
                            # pre-satisfied, so even/odd couples co-issue and
                            # col-tile on the PE); strictly paired emission
                            while st["odone"] + 1 < st["ready"] - 6:
                                emit_o(st)
                                emit_o(st)
                    while st["odone"] < MC:
                        emit_o(st)
                    prev = st
                tail_part1(prev)
                tail_part2(prev)
                tail_part3(prev)
                psA.release()
                psO_p.release()
                psS.release()
                if DBG:
                    dp = outpool.tile([72, M], f32, name="dp", tag="dbg3", bufs=1)
                    nc.vector.tensor_copy(dp, phi_sb[:])
                    nc.sync.dma_start(out=dbg_phi[:], in_=dp)
                    for hw in range(4):
                        dt_ = outpool.tile([96, NS // 4], f32, name="dt_", tag="dbg4", bufs=1)
                        nc.vector.tensor_copy(dt_, theta_sb[:, hw * 2048 : (hw + 1) * 2048])
                        nc.sync.dma_start(out=dbg_th[:, hw * 2048 : (hw + 1) * 2048], in_=dt_)
                    dg = outpool.tile([128, MC * 64], f32, name="dg", tag="dbg5", bufs=1)
                    nc.vector.tensor_copy(dg, gtv[:])
                    nc.sync.dma_start(out=dbg_gt[:], in_=dg)



    nc.finalize()
    return nc


def _maybe_trace_setup():
    """Optional NTFF profiling (test harness only, via NLATTN_TRACE=1)."""
    if not os.environ.get("NLATTN_TRACE"):
        return False
    import types

    try:
        from antenv.axon_hooks import get_axon_ntff_profile_hook  # noqa: F401
    except ImportError:
        import antenv

        mod = types.ModuleType("antenv.axon_hooks")
        mod._hook = None

        def set_axon_ntff_profile_hook(h):
            mod._hook = h

        def get_axon_ntff_profile_hook():
            return mod._hook

        mod.set_axon_ntff_profile_hook = set_axon_ntff_profile_hook
        mod.get_axon_ntff_profile_hook = get_axon_ntff_profile_hook
        sys.modules["antenv.axon_hooks"] = mod
        antenv.axon_hooks = mod
        from trn_agent_boot.trn_boot import _ntff_profile_via_ctypes

        mod._hook = _ntff_profile_via_ctypes("/opt/axon/libaxon_pjrt.so")
    import concourse.bass_utils as bu

    bu.upload_artifacts = lambda tmpdir: "local://" + str(tmpdir)
    return True


_LAST_RESULT = {}


def kernel(x, w_theta, w_phi, w_g, w_o, gamma):
    from concourse.bass_utils import run_bass_kernel_spmd

    trace = _maybe_trace_setup()

    B = np.asarray(x).shape[0]
    xf = np.ascontiguousarray(np.asarray(x).reshape(B, C, N), dtype=np.float32)
    w_pg_h = np.ascontiguousarray(
        np.concatenate(
            [np.asarray(w_g), np.asarray(w_phi), np.zeros((24, C), np.float32)],
            axis=0,
        ).T,
        dtype=np.float32,
    )
    w_th_h = np.ascontiguousarray(
        np.concatenate([np.asarray(w_theta), np.zeros((24, C), np.float32)], axis=0).T,
        dtype=np.float32,
    )
    w_oT_h = np.ascontiguousarray(np.asarray(w_o).T, dtype=np.float32)
    gamma_h = np.asarray(gamma, dtype=np.float32).reshape(1, 1)

    nc = _build_program()

    in_maps = []
    for core in range(8):
        b, s = core // 4, core % 4
        in_maps.append(
            {
                "x_full": xf[b],
                "x_slice": np.ascontiguousarray(xf[b][:, s * NS : (s + 1) * NS]),
                "w_pg": w_pg_h,
                "w_th": w_th_h,
                "w_oT": w_oT_h,
                "gamma": gamma_h,
            }
        )

    res = run_bass_kernel_spmd(nc, in_maps, core_ids=list(range(8)), trace=trace)
    _LAST_RESULT["exec_time_ns"] = res.exec_time_ns
    _LAST_RESULT["trace"] = res.instructions_and_trace

    out = np.empty((B, C, N), dtype=np.float32)
    for core in range(8):
        b, s = core // 4, core % 4
        out[b][:, s * NS : (s + 1) * NS] = res.results[core]["out"]
    D = H = W = 32
    return out.reshape(B, C, D, H, W)


# revision 24
# speedup vs baseline: 1.0429x; 1.0429x over previous
"""Trainium2 Bass kernel for a 3D non-local attention block.

Math (per batch b):
  xf = x.reshape(C, N)                         C=64, N=32768 (=32^3)
  theta = w_theta @ xf                         [8, N]
  phi   = maxpool2(w_phi @ xf)                 [8, M], M=4096
  g     = maxpool2(w_g   @ xf)                 [32, M]
  beta  = softmax_over_m(theta^T phi)          [N, M]
  o     = g @ beta^T                           [32, N]
  out   = gamma * (w_o @ o) + xf               [C, N]

Sharding: 8 cores, core k -> batch k//4, query slice k%4 (8192 queries).
Every core re-computes the (cheap) pooled phi/g from the full batch and
runs flash-style attention over its own query slice; no collectives.

v2: the exp(S) stream (33.5M elems/core, the hard floor at ~1 elem/
cycle/lane on ScalarE) is split across TWO engines: ScalarE runs table
exp on ~2/3 of the m-chunk groups and the DVE runs a Schraudolph
bit-trick exp (int16(A*x+B) bitcast to bf16) on the rest.  Pooling is
restructured so projection matmuls col-tile two 512-col x-chunks into
one [128,512] PSUM bank and the 2x2x2 maxpool runs as strided TT max
(fp32 PSUM) then 2x-mode bf16 TTs.  G' is built with DMA transposes (no
PE/ScalarE cost).  The o-matmuls accumulate even/odd chunks into ONE
PSUM bank (partitions 0:64 / 64:128) and the softmax denominator merge
folds into a single [33,F] TT.  PSUM: psS 2x3 banks + psO 1 + psA 1 = 8.
"""

import os
import sys

sys.path.insert(0, "/opt/trn_rl_repo")

import numpy as np

C = 64            # channels
N = 32768         # voxels (32^3)
NS = N // 4       # query slice per core (8192)
M = N // 8        # pooled keys (4096)
F = 512           # free-dim tile (PSUM bank)
NT = NS // F      # 16 n-tiles per core
MC = M // 128     # 32 m-chunks of 128
GROUPS = [(s, min(s + 3, MC)) for s in range(0, MC, 3)]  # 11 groups (last=2)
NG = len(GROUPS)

# Schraudolph constants: exp(x) ~ bitcast_bf16(int16(A16*x + B16))
A16 = float(2.0**7 / np.log(2.0))
B16 = float(127.0 * 2.0**7 - 366393.0 / 65536.0)

# Which exp groups go to the DVE (Schraudolph) per tile. Tiles 0-1 are
# all-ScalarE so the DVE can finish the phase-A pooling backlog.
DVE_GROUPS_EVEN = (2, 5, 8)
DVE_GROUPS_ODD = (2, 6, 9)


def _dve_groups(t):
    if os.environ.get("NLATTN_NO_DVE_EXP"):
        return ()
    if t < 2:
        return (5, 9)
    return DVE_GROUPS_EVEN if t % 2 == 0 else DVE_GROUPS_ODD


def _build_program():
    import concourse.bass as bass  # noqa: F401
    import concourse.tile as tile
    from concourse import bacc, mybir

    f32 = mybir.dt.float32
    f32r = mybir.dt.float32r
    bf16 = mybir.dt.bfloat16
    fp16 = mybir.dt.float16
    i16 = mybir.dt.int16

    nc = bacc.Bacc()

    x_full = nc.declare_dram_parameter("x_full", [C, N], f32, isOutput=False)
    x_slice = nc.declare_dram_parameter("x_slice", [C, NS], f32, isOutput=False)
    w_pg = nc.declare_dram_parameter("w_pg", [C, 64], f32, isOutput=False)
    w_th = nc.declare_dram_parameter("w_th", [C, 32], f32, isOutput=False)
    w_oT = nc.declare_dram_parameter("w_oT", [32, C], f32, isOutput=False)
    gamma = nc.declare_dram_parameter("gamma", [1, 1], f32, isOutput=False)
    out_d = nc.declare_dram_parameter("out", [C, NS], f32, isOutput=True)
    DBG = bool(os.environ.get("NLATTN_DEBUG"))
    if DBG:
        dbg_phi = nc.declare_dram_parameter("dbg_phi", [72, M], f32, isOutput=True)
        dbg_th = nc.declare_dram_parameter("dbg_th", [96, NS], f32, isOutput=True)
        dbg_gt = nc.declare_dram_parameter("dbg_gt", [128, MC * 64], f32, isOutput=True)
        dbg_o33 = nc.declare_dram_parameter("dbg_o33", [33, F], f32, isOutput=True)
        dbg_den = nc.declare_dram_parameter("dbg_den", [1, F], f32, isOutput=True)
        dbg_exp = nc.declare_dram_parameter("dbg_exp", [128, MC, F], f32, isOutput=True)

    Exp = mybir.ActivationFunctionType.Exp
    Max = mybir.AluOpType.max
    Add = mybir.AluOpType.add
    Mult = mybir.AluOpType.mult

    with tile.TileContext(nc) as tc:
        with (
            tc.tile_pool(name="consts", bufs=1) as consts,
            tc.tile_pool(name="big", bufs=2) as bigpool,
            tc.tile_pool(name="s1p", bufs=2) as s1pool,
            tc.tile_pool(name="s2p", bufs=2) as s2pool,
            tc.tile_pool(name="gpp", bufs=2) as gppool,
            tc.tile_pool(name="theta", bufs=1) as thpool,
            tc.tile_pool(name="pg", bufs=1) as pgpool,
            tc.tile_pool(name="gtp", bufs=1) as gtpool,
            tc.tile_pool(name="th4p", bufs=2) as th4pool,
            tc.tile_pool(name="xin", bufs=3) as xpool,
            tc.tile_pool(name="small", bufs=2) as smallpool,
            tc.tile_pool(name="outp", bufs=2) as outpool,
        ):
            psS = tc.alloc_tile_pool(name="psS", bufs=2, space="PSUM")
            psO_p = tc.alloc_tile_pool(name="psO", bufs=1, space="PSUM")
            psA = tc.alloc_tile_pool(name="psA", bufs=1, space="PSUM")
            w_pg_sb = consts.tile([C, 64], fp16)
            nc.gpsimd.dma_start(out=w_pg_sb, in_=w_pg[:])
            w_th_sb = consts.tile([C, 32], fp16)
            nc.gpsimd.dma_start(out=w_th_sb, in_=w_th[:])
            gamma_sb = consts.tile([1, 1], f32)
            nc.sync.dma_start(out=gamma_sb, in_=gamma[:])
            w_oT_f32 = consts.tile([32, C], f32)
            nc.sync.dma_start(out=w_oT_f32, in_=w_oT[:])
            w_oT_sb = consts.tile([32, C], f32r)
            g32 = consts.tile([32, 1], f32)
            nc.gpsimd.partition_broadcast(g32, gamma_sb)
            nc.vector.tensor_scalar_mul(w_oT_sb, w_oT_f32, g32)
            ones32 = consts.tile([128, 32], f32)
            nc.vector.memset(ones32, 1.0)
            zeros_sb = consts.tile([128, F], f32)
            nc.vector.memset(zeros_sb, 0.0)

            # HAM warm-up: ~6us of dense back-to-back matmuls flips the PE
            # clock gate from 4/8 (1.2 GHz) to 8/8 (2.4 GHz); the steady-state
            # pipeline never idles the PE >3.4us, so it stays warm after.
            prim_w = consts.tile([64, 64], bf16)
            nc.vector.memset(prim_w, 0.0)
            prim_x = consts.tile([64, F], bf16)
            nc.vector.memset(prim_x, 0.0)
            prim_ps = psO_p.tile([128, F], f32, name="prim_ps", tag="psO")
            for _ in range(14):
                nc.tensor.matmul(
                    prim_ps[0:64, :], prim_w, prim_x, start=True, stop=True
                )

            # phi replicas at partition offsets 0/32/64 for row-tiled S.
            phi_sb = pgpool.tile([72, M], bf16)
            theta_sb = thpool.tile([96, NS], bf16, tag="th96")

            # G' = [g^T | 1], zero-padded to 64 columns, chunk-major.
            gt = gtpool.tile([128, MC, 64], bf16)
            gtv = gt.rearrange("p a b -> p (a b)")
            for z0 in range(0, MC * 64, F):
                nc.scalar.copy(gtv[:, z0 : z0 + F], zeros_sb[:, 0:F])
            nc.scalar.copy(gt[:, :, 32], ones32)

            # ---------------- PSUM pools (shared by phase A and C) -----------
            def phase_a_quarter_steps(q):
                """Emission callbacks for quarter q's projection + 2x2x2
                maxpool; lazily allocates its tiles at first call."""
                st = {}
                m0 = q * 1024

                def fill(p):
                    # one [128, F] psA bank <- 1024 x-cols (one d-slice piece,
                    # h-halves on the partition axis)
                    def go():
                        if p == 0:
                            st["s1"] = s1pool.tile([128, 2048], bf16, tag="s1", name="s1")
                            st["s2"] = s2pool.tile([128, 1024], bf16, tag="s2", name="s2")
                            st["gp"] = gppool.tile([128, 512], bf16, tag="gp", name="gp")
                        if p % 2 == 0:
                            xc = xpool.tile([C, 2048], fp16, tag="x")
                            base = q * 8192 + (p // 2) * 2048
                            nc.gpsimd.dma_start(out=xc, in_=x_full[:, base : base + 2048])
                            st["xc"] = xc
                        xc = st["xc"]
                        u = (p % 2) * 1024
                        ps = psA.tile([128, F], f32, tag="pgbank", name="psAfill")
                        nc.tensor.matmul(
                            ps[0:64, :], w_pg_sb, xc[:, u : u + 512],
                            start=True, stop=True, tile_position=(0, 0),
                        )
                        nc.tensor.matmul(
                            ps[64:128, :], w_pg_sb, xc[:, u + 512 : u + 1024],
                            start=True, stop=True, tile_position=(0, 64),
                        )
                        nc.vector.tensor_reduce(
                            st["s1"][:, p * 256 : (p + 1) * 256],
                            ps.rearrange("c (a two) -> c a two", two=2),
                            mybir.AxisListType.X,
                            Max,
                        )
                    return go

                for p in range(8):
                    yield fill(p)

                def hpool():
                    v = st["s1"].rearrange(
                        "c (p j two w) -> c p j two w", p=8, j=8, two=2, w=16
                    )
                    nc.vector.tensor_tensor(
                        st["s2"], v[:, :, :, 0, :], v[:, :, :, 1, :], Max
                    )
                yield hpool

                def dpool():
                    v = st["s2"].rearrange(
                        "c (pp two r) -> c pp two r", pp=4, two=2, r=128
                    )
                    nc.vector.tensor_tensor(st["gp"], v[:, :, 0, :], v[:, :, 1, :], Max)
                yield dpool

                def shuffle():
                    gp = st["gp"]
                    # phi rows (32:40 / 96:104 of gp) -> phi_sb replicas.
                    # gp free=(pp,j,w): local m = pp*256 + hhalf*128 + j*16 + w
                    for hh in range(2):
                        src = gp[hh * 64 + 32 : hh * 64 + 40, :].rearrange(
                            "c (pp r) -> c pp r", pp=4
                        )
                        for off in (0, 32, 64):
                            dst = phi_sb[off : off + 8, m0 : m0 + 1024].rearrange(
                                "c (pp two r) -> c pp two r", pp=4, two=2
                            )
                            nc.scalar.dma_start(out=dst[:, :, hh, :], in_=src)
                    # G' chunks via DMA transpose: chunk (q*8 + pp*2 + hh)
                    for cl in range(8):
                        pp, hh = cl // 2, cl % 2
                        nc.sync.dma_start(
                            out=gt[:, q * 8 + cl, 0:32],
                            in_=gp[hh * 64 : hh * 64 + 32, pp * 128 : (pp + 1) * 128],
                            transpose=True,
                        )
                yield shuffle

            def theta_steps():
                """theta projection from x_slice: 4 bank-fills, each 4-way
                col-tiled (2048 slice-cols per bank), ScalarE evacuation."""
                def fill(tf):
                    def go():
                        xt = xpool.tile([C, 2048], fp16, tag="x")
                        nc.gpsimd.dma_start(
                            out=xt, in_=x_slice[:, tf * 2048 : (tf + 1) * 2048]
                        )
                        th = psA.tile([128, F], f32, tag="pgbank", name="psThfill")
                        for c in range(4):
                            nc.tensor.matmul(
                                th[32 * c : 32 * c + 32, :],
                                w_th_sb,
                                xt[:, c * 512 : (c + 1) * 512],
                                start=True, stop=True, tile_position=(0, 32 * c),
                            )
                        th4 = th4pool.tile([128, F], bf16, tag="th4")
                        nc.scalar.copy(th4, th)
                        for c in range(4):
                            nc.scalar.dma_start(
                                out=theta_sb[
                                    0:8, tf * 2048 + c * 512 : tf * 2048 + (c + 1) * 512
                                ],
                                in_=th4[32 * c : 32 * c + 8, :],
                            )
                    return go

                for tf in range(4):
                    yield fill(tf)

                def replicate():
                    for off in (32, 64):
                        nc.scalar.dma_start(
                            out=theta_sb[off : off + 8, :], in_=theta_sb[0:8, :]
                        )
                yield replicate

            a_steps = list(theta_steps())
            for q in range(4):
                a_steps.extend(phase_a_quarter_steps(q))
            a_idx = 0

            def drain_a(k):
                nonlocal a_idx
                for _ in range(k):
                    if a_idx < len(a_steps):
                        a_steps[a_idx]()
                        a_idx += 1

            # Emit theta + quarter 0 up front; quarters 1-3 interleave below.
            drain_a(5 + 11)

            # ---------------- Phase C: flash attention -----------------------
            if True:
                def make_state(t):
                    return {
                        "t": t,
                        "n0": t * F,
                        "expS": bigpool.tile([128, MC, F], bf16, tag="big", name="expS"),
                        "psO": psO_p.tile([128, F], f32, name="psO", tag="psO"),
                        "ready": 0,
                        "odone": 0,
                        "ne": 0,
                        "no": 0,
                    }

                def emit_group(st, gi):
                    mc0, mc1 = GROUPS[gi]
                    cnt = mc1 - mc0
                    sps = psS.tile([128, 3 * F], f32, tag="psS", name="sps")
                    for i, mc in enumerate(range(mc0, mc1)):
                        nc.tensor.matmul(
                            sps[:, i * F : (i + 1) * F],
                            phi_sb[32 * i : 32 * i + 8, mc * 128 : (mc + 1) * 128],
                            theta_sb[32 * i : 32 * i + 8, st["n0"] : st["n0"] + F],
                            start=True,
                            stop=True,
                            tile_position=(32 * i, 0),
                        )
                    if gi in _dve_groups(st["t"]):
                        nc.vector.tensor_scalar(
                            st["expS"][:, mc0:mc1, :].bitcast(i16),
                            sps[:, 0 : cnt * F],
                            A16,
                            B16,
                            Mult,
                            Add,
                        )
                    else:
                        nc.scalar.activation(
                            out=st["expS"][:, mc0:mc1, :],
                            in_=sps[:, 0 : cnt * F],
                            func=Exp,
                        )
                    st["ready"] = mc1

                def emit_o(st):
                    mc = st["odone"]
                    par = mc % 2
                    first = (st["ne"] if par == 0 else st["no"]) == 0
                    last = (st["ne"] if par == 0 else st["no"]) == MC // 2 - 1
                    nc.tensor.matmul(
                        st["psO"][0:64, :] if par == 0 else st["psO"][64:128, :],
                        gt[:, mc, :],
                        st["expS"][:, mc, :],
                        start=first,
                        stop=last,
                        tile_position=(0, 0) if par == 0 else (0, 64),
                    )
                    if par == 0:
                        st["ne"] += 1
                    else:
                        st["no"] += 1
                    st["odone"] += 1

                def tail_part1(st):
                    # merge even/odd sums; separate den TT (the custom-DVE
                    # reciprocal requires a base-0 in-place operand)
                    psO = st["psO"]
                    tmp = smallpool.tile([33, F], f32, tag="ob")
                    nc.vector.tensor_copy(tmp, psO[64:97, :])
                    o32 = smallpool.tile([32, F], f32r, tag="o33")
                    nc.vector.tensor_tensor(o32, psO[0:32, :], tmp[0:32, :], Add)
                    den = smallpool.tile([1, F], f32, tag="den")
                    nc.vector.tensor_tensor(den, psO[32:33, :], tmp[32:33, :], Add)
                    st["o32"] = o32
                    st["den"] = den
                    if DBG and st["t"] == 0:
                        d1 = outpool.tile([33, F], f32, name="d1", tag="dbg1", bufs=1)
                        nc.vector.tensor_copy(d1[0:32], o32[:].bitcast(f32))
                        nc.vector.tensor_copy(d1[32:33], den)
                        nc.sync.dma_start(out=dbg_o33[:], in_=d1)
                        for pc in range(4):
                            d2 = outpool.tile([128, MC // 4, F], f32, name="d2", tag="dbg2", bufs=1)
                            nc.vector.tensor_copy(d2, st["expS"][:, pc * 8 : (pc + 1) * 8, :])
                            nc.sync.dma_start(out=dbg_exp[:, pc * 8 : (pc + 1) * 8, :], in_=d2)
                    xres = xpool.tile([C, F], f32, tag="xres", bufs=2)
                    nc.gpsimd.dma_start(out=xres, in_=x_slice[:, st["n0"] : st["n0"] + F])
                    st["xres"] = xres

                def tail_part2(st):
                    den = st["den"]
                    nc.vector.reciprocal_approx_fast(out=den, in_=den)
                    rb = smallpool.tile([64, F], f32, tag="rb")
                    nc.gpsimd.partition_broadcast(rb, den)
                    st["rb"] = rb
                    if DBG and st["t"] == 0:
                        nc.sync.dma_start(out=dbg_den[:], in_=den)
                    # project unnormalized o; (w_o@o)/den == w_o@(o/den)
                    nc.tensor.matmul(
                        st["psO"][0:64, :],
                        w_oT_sb,
                        st["o32"],
                        start=True,
                        stop=True,
                    )

                def tail_part3(st):
                    pn = smallpool.tile([64, F], f32, tag="pn")
                    nc.vector.tensor_mul(pn, st["psO"][0:64, :], st["rb"])
                    ot = outpool.tile([C, F], f32)
                    nc.vector.tensor_add(ot, pn, st["xres"])
                    nc.gpsimd.dma_start(out=out_d[:, st["n0"] : st["n0"] + F], in_=ot)

                prev = None
                for t in range(NT):
                    st = make_state(t)
                    for gi in range(NG):
                        drain_a(4 if t < 2 else 2)
                        emit_group(st, gi)
                        if prev is not None:
                            # tails early: the single psO bank must be fully
                            # read before this tile's o-matmuls start
                            if gi == 0:
                                tail_part1(prev)
                            elif gi == 1:
                                tail_part2(prev)
                            elif gi == 2:
                                tail_part3(prev)
                                prev = None
                        else:
                            # lag o-matmuls ~2 groups behind exp (sems are
                            # pre-satisfied, so even/odd couples co-issue and
                            # col-tile on the PE); strictly paired emission
                            while st["odone"] + 1 < st["ready"] - 6:
                                emit_o(st)
                                emit_o(st)
                    while st["odone"] < MC:
                        emit_o(st)
                    prev = st
                tail_part1(prev)
                tail_part2(prev)
                tail_part3(prev)
                psA.release()
                psO_p.release()
                psS.release()
                if DBG:
                    dp = outpool.tile([72, M], f32, name="dp", tag="dbg3", bufs=1)
                    nc.vector.tensor_copy(dp, phi_sb[:])
                    nc.sync.dma_start(out=dbg_phi[:], in_=dp)
                    for hw in range(4):
                        dt_ = outpool.tile([96, NS // 4], f32, name="dt_", tag="dbg4", bufs=1)
                        nc.vector.tensor_copy(dt_, theta_sb[:, hw * 2048 : (hw + 1) * 2048])
                        nc.sync.dma_start(out=dbg_th[:, hw * 2048 : (hw + 1) * 2048], in_=dt_)
                    dg = outpool.tile([128, MC * 64], f32, name="dg", tag="dbg5", bufs=1)
                    nc.vector.tensor_copy(dg, gtv[:])
                    nc.sync.dma_start(out=dbg_gt[:], in_=dg)



    nc.finalize()
    return nc


def _maybe_trace_setup():
    """Optional NTFF profiling (test harness only, via NLATTN_TRACE=1)."""
    if not os.environ.get("NLATTN_TRACE"):
        return False
    import types

    try:
        from antenv.axon_hooks import get_axon_ntff_profile_hook  # noqa: F401
    except ImportError:
        import antenv

        mod = types.ModuleType("antenv.axon_hooks")
        mod._hook = None

        def set_axon_ntff_profile_hook(h):
            mod._hook = h

        def get_axon_ntff_profile_hook():
            return mod._hook

        mod.set_axon_ntff_profile_hook = set_axon_ntff_profile_hook
        mod.get_axon_ntff_profile_hook = get_axon_ntff_profile_hook
        sys.modules["antenv.axon_hooks"] = mod
        antenv.axon_hooks = mod
        from trn_agent_boot.trn_boot import _ntff_profile_via_ctypes

        mod._hook = _ntff_profile_via_ctypes("/opt/axon/libaxon_pjrt.so")
    import concourse.bass_utils as bu

    bu.upload_artifacts = lambda tmpdir: "local://" + str(tmpdir)
    return True


_LAST_RESULT = {}


def kernel(x, w_theta, w_phi, w_g, w_o, gamma):
    from concourse.bass_utils import run_bass_kernel_spmd

    trace = _maybe_trace_setup()

    B = np.asarray(x).shape[0]
    xf = np.ascontiguousarray(np.asarray(x).reshape(B, C, N), dtype=np.float32)
    w_pg_h = np.ascontiguousarray(
        np.concatenate(
            [np.asarray(w_g), np.asarray(w_phi), np.zeros((24, C), np.float32)],
            axis=0,
        ).T,
        dtype=np.float32,
    )
    w_th_h = np.ascontiguousarray(
        np.concatenate([np.asarray(w_theta), np.zeros((24, C), np.float32)], axis=0).T,
        dtype=np.float32,
    )
    w_oT_h = np.ascontiguousarray(np.asarray(w_o).T, dtype=np.float32)
    gamma_h = np.asarray(gamma, dtype=np.float32).reshape(1, 1)

    nc = _build_program()

    in_maps = []
    for core in range(8):
        b, s = core // 4, core % 4
        in_maps.append(
            {
                "x_full": xf[b],
                "x_slice": np.ascontiguousarray(xf[b][:, s * NS : (s + 1) * NS]),
                "w_pg": w_pg_h,
                "w_th": w_th_h,
                "w_oT": w_oT_h,
                "gamma": gamma_h,
            }
        )

    res = run_bass_kernel_spmd(nc, in_maps, core_ids=list(range(8)), trace=trace)
    _LAST_RESULT["exec_time_ns"] = res.exec_time_ns
    _LAST_RESULT["trace"] = res.instructions_and_trace

    out = np.empty((B, C, N), dtype=np.float32)
    for core in range(8):
        b, s = core // 4, core % 4
        out[b][:, s * NS : (s + 1) * NS] = res.results[core]["out"]
    D = H = W = 32
    return out.reshape(B, C, D, H, W)


# revision 25
# speedup vs baseline: 1.2970x; 1.2436x over previous
"""Trainium2 Bass kernel for a 3D non-local attention block.

Math (per batch b):
  xf = x.reshape(C, N)                         C=64, N=32768 (=32^3)
  theta = w_theta @ xf                         [8, N]
  phi   = maxpool2(w_phi @ xf)                 [8, M], M=4096
  g     = maxpool2(w_g   @ xf)                 [32, M]
  beta  = softmax_over_m(theta^T phi)          [N, M]
  o     = g @ beta^T                           [32, N]
  out   = gamma * (w_o @ o) + xf               [C, N]

Sharding: 8 cores, core k -> batch k//4, query slice k%4 (8192 queries).
Every core re-computes the (cheap) pooled phi/g from the full batch and
runs flash-style attention over its own query slice; no collectives.

v2: the exp(S) stream (33.5M elems/core, the hard floor at ~1 elem/
cycle/lane on ScalarE) is split across TWO engines: ScalarE runs table
exp on ~2/3 of the m-chunk groups and the DVE runs a Schraudolph
bit-trick exp (int16(A*x+B) bitcast to bf16) on the rest.  Pooling is
restructured so projection matmuls col-tile two 512-col x-chunks into
one [128,512] PSUM bank and the 2x2x2 maxpool runs as strided TT max
(fp32 PSUM) then 2x-mode bf16 TTs.  G' is built with DMA transposes (no
PE/ScalarE cost).  The o-matmuls accumulate even/odd chunks into ONE
PSUM bank (partitions 0:64 / 64:128) and the softmax denominator merge
folds into a single [33,F] TT.  PSUM: psS 2x3 banks + psO 1 + psA 1 = 8.
"""

import os
import sys

sys.path.insert(0, "/opt/trn_rl_repo")

import numpy as np

C = 64            # channels
N = 32768         # voxels (32^3)
NS = N // 4       # query slice per core (8192)
M = N // 8        # pooled keys (4096)
F = 512           # free-dim tile (PSUM bank)
NT = NS // F      # 16 n-tiles per core
MC = M // 128     # 32 m-chunks of 128
GROUPS = [(s, min(s + 3, MC)) for s in range(0, MC, 3)]  # 11 groups (last=2)
NG = len(GROUPS)

# Schraudolph constants: exp(x) ~ bitcast_bf16(int16(A16*x + B16))
A16 = float(2.0**7 / np.log(2.0))
B16 = float(127.0 * 2.0**7 - 366393.0 / 65536.0)

# Which exp groups go to the DVE (Schraudolph) per tile. Tiles 0-1 are
# all-ScalarE so the DVE can finish the phase-A pooling backlog.
DVE_GROUPS_EVEN = (2, 5, 8)
DVE_GROUPS_ODD = (1, 4, 7, 9)


def _dve_groups(t):
    if os.environ.get("NLATTN_NO_DVE_EXP"):
        return ()
    if t < 2:
        return (5, 9)
    return DVE_GROUPS_EVEN if t % 2 == 0 else DVE_GROUPS_ODD


def _build_program():
    import concourse.bass as bass  # noqa: F401
    import concourse.tile as tile
    from concourse import bacc, mybir

    f32 = mybir.dt.float32
    f32r = mybir.dt.float32r
    bf16 = mybir.dt.bfloat16
    fp16 = mybir.dt.float16
    i16 = mybir.dt.int16

    nc = bacc.Bacc()

    x_full = nc.declare_dram_parameter("x_full", [C, N], f32, isOutput=False)
    x_slice = nc.declare_dram_parameter("x_slice", [C, NS], f32, isOutput=False)
    w_pg = nc.declare_dram_parameter("w_pg", [C, 64], f32, isOutput=False)
    w_th = nc.declare_dram_parameter("w_th", [C, 32], f32, isOutput=False)
    w_oT = nc.declare_dram_parameter("w_oT", [32, C], f32, isOutput=False)
    gamma = nc.declare_dram_parameter("gamma", [1, 1], f32, isOutput=False)
    out_d = nc.declare_dram_parameter("out", [C, NS], f32, isOutput=True)
    DBG = bool(os.environ.get("NLATTN_DEBUG"))
    if DBG:
        dbg_phi = nc.declare_dram_parameter("dbg_phi", [72, M], f32, isOutput=True)
        dbg_th = nc.declare_dram_parameter("dbg_th", [96, NS], f32, isOutput=True)
        dbg_gt = nc.declare_dram_parameter("dbg_gt", [128, MC * 64], f32, isOutput=True)
        dbg_o33 = nc.declare_dram_parameter("dbg_o33", [33, F], f32, isOutput=True)
        dbg_den = nc.declare_dram_parameter("dbg_den", [1, F], f32, isOutput=True)
        dbg_exp = nc.declare_dram_parameter("dbg_exp", [128, MC, F], f32, isOutput=True)

    Exp = mybir.ActivationFunctionType.Exp
    Max = mybir.AluOpType.max
    Add = mybir.AluOpType.add
    Mult = mybir.AluOpType.mult

    with tile.TileContext(nc) as tc:
        with (
            tc.tile_pool(name="consts", bufs=1) as consts,
            tc.tile_pool(name="big", bufs=2) as bigpool,
            tc.tile_pool(name="s1p", bufs=2) as s1pool,
            tc.tile_pool(name="s2p", bufs=2) as s2pool,
            tc.tile_pool(name="gpp", bufs=2) as gppool,
            tc.tile_pool(name="theta", bufs=1) as thpool,
            tc.tile_pool(name="pg", bufs=1) as pgpool,
            tc.tile_pool(name="gtp", bufs=1) as gtpool,
            tc.tile_pool(name="th4p", bufs=2) as th4pool,
            tc.tile_pool(name="xin", bufs=3) as xpool,
            tc.tile_pool(name="small", bufs=2) as smallpool,
            tc.tile_pool(name="outp", bufs=2) as outpool,
        ):
            psS = tc.alloc_tile_pool(name="psS", bufs=2, space="PSUM")
            psO_p = tc.alloc_tile_pool(name="psO", bufs=1, space="PSUM")
            psA = tc.alloc_tile_pool(name="psA", bufs=1, space="PSUM")
            w_pg_sb = consts.tile([C, 64], fp16)
            nc.gpsimd.dma_start(out=w_pg_sb, in_=w_pg[:])
            w_th_sb = consts.tile([C, 32], fp16)
            nc.gpsimd.dma_start(out=w_th_sb, in_=w_th[:])
            gamma_sb = consts.tile([1, 1], f32)
            nc.sync.dma_start(out=gamma_sb, in_=gamma[:])
            w_oT_f32 = consts.tile([32, C], f32)
            nc.sync.dma_start(out=w_oT_f32, in_=w_oT[:])
            w_oT_sb = consts.tile([32, C], f32r)
            g32 = consts.tile([32, 1], f32)
            nc.gpsimd.partition_broadcast(g32, gamma_sb)
            nc.vector.tensor_scalar_mul(w_oT_sb, w_oT_f32, g32)
            ones32 = consts.tile([128, 32], f32)
            nc.vector.memset(ones32, 1.0)
            zeros_sb = consts.tile([128, F], f32)
            nc.vector.memset(zeros_sb, 0.0)

            # HAM warm-up: ~6us of dense back-to-back matmuls flips the PE
            # clock gate from 4/8 (1.2 GHz) to 8/8 (2.4 GHz); the steady-state
            # pipeline never idles the PE >3.4us, so it stays warm after.
            prim_w = consts.tile([64, 64], bf16)
            nc.vector.memset(prim_w, 0.0)
            prim_x = consts.tile([64, F], bf16)
            nc.vector.memset(prim_x, 0.0)
            prim_ps = psO_p.tile([128, F], f32, name="prim_ps", tag="psO")
            for _ in range(14):
                nc.tensor.matmul(
                    prim_ps[0:64, :], prim_w, prim_x, start=True, stop=True
                )

            # phi replicas at partition offsets 0/32/64 for row-tiled S.
            phi_sb = pgpool.tile([72, M], bf16)
            theta_sb = thpool.tile([96, NS], bf16, tag="th96")

            # G' = [g^T | 1], zero-padded to 64 columns, chunk-major.
            gt = gtpool.tile([128, MC, 64], bf16)
            gtv = gt.rearrange("p a b -> p (a b)")
            for z0 in range(0, MC * 64, F):
                nc.scalar.copy(gtv[:, z0 : z0 + F], zeros_sb[:, 0:F])
            nc.scalar.copy(gt[:, :, 32], ones32)

            # ---------------- PSUM pools (shared by phase A and C) -----------
            def phase_a_quarter_steps(q):
                """Emission callbacks for quarter q's projection + 2x2x2
                maxpool; lazily allocates its tiles at first call."""
                st = {}
                m0 = q * 1024

                def fill(p):
                    # one [128, F] psA bank <- 1024 x-cols (one d-slice piece,
                    # h-halves on the partition axis)
                    def go():
                        if p == 0:
                            st["s1"] = s1pool.tile([128, 2048], bf16, tag="s1", name="s1")
                            st["s2"] = s2pool.tile([128, 1024], bf16, tag="s2", name="s2")
                            st["gp"] = gppool.tile([128, 512], bf16, tag="gp", name="gp")
                        if p % 2 == 0:
                            xc = xpool.tile([C, 2048], fp16, tag="x")
                            base = q * 8192 + (p // 2) * 2048
                            nc.gpsimd.dma_start(out=xc, in_=x_full[:, base : base + 2048])
                            st["xc"] = xc
                        xc = st["xc"]
                        u = (p % 2) * 1024
                        ps = psA.tile([128, F], f32, tag="pgbank", name="psAfill")
                        nc.tensor.matmul(
                            ps[0:64, :], w_pg_sb, xc[:, u : u + 512],
                            start=True, stop=True, tile_position=(0, 0),
                        )
                        nc.tensor.matmul(
                            ps[64:128, :], w_pg_sb, xc[:, u + 512 : u + 1024],
                            start=True, stop=True, tile_position=(0, 64),
                        )
                        nc.vector.tensor_reduce(
                            st["s1"][:, p * 256 : (p + 1) * 256],
                            ps.rearrange("c (a two) -> c a two", two=2),
                            mybir.AxisListType.X,
                            Max,
                        )
                    return go

                for p in range(8):
                    yield fill(p)

                def hpool():
                    v = st["s1"].rearrange(
                        "c (p j two w) -> c p j two w", p=8, j=8, two=2, w=16
                    )
                    nc.vector.tensor_tensor(
                        st["s2"], v[:, :, :, 0, :], v[:, :, :, 1, :], Max
                    )
                yield hpool

                def dpool():
                    v = st["s2"].rearrange(
                        "c (pp two r) -> c pp two r", pp=4, two=2, r=128
                    )
                    nc.vector.tensor_tensor(st["gp"], v[:, :, 0, :], v[:, :, 1, :], Max)
                yield dpool

                def shuffle():
                    gp = st["gp"]
                    # phi rows (32:40 / 96:104 of gp) -> phi_sb replicas.
                    # gp free=(pp,j,w): local m = pp*256 + hhalf*128 + j*16 + w
                    for hh in range(2):
                        src = gp[hh * 64 + 32 : hh * 64 + 40, :].rearrange(
                            "c (pp r) -> c pp r", pp=4
                        )
                        for off in (0, 32, 64):
                            dst = phi_sb[off : off + 8, m0 : m0 + 1024].rearrange(
                                "c (pp two r) -> c pp two r", pp=4, two=2
                            )
                            nc.sync.dma_start(out=dst[:, :, hh, :], in_=src)
                    # G' chunks via DMA transpose: chunk (q*8 + pp*2 + hh)
                    for cl in range(8):
                        pp, hh = cl // 2, cl % 2
                        nc.sync.dma_start(
                            out=gt[:, q * 8 + cl, 0:32],
                            in_=gp[hh * 64 : hh * 64 + 32, pp * 128 : (pp + 1) * 128],
                            transpose=True,
                        )
                yield shuffle

            def theta_steps():
                """theta projection from x_slice: 4 bank-fills, each 4-way
                col-tiled (2048 slice-cols per bank), ScalarE evacuation."""
                def fill(tf):
                    def go():
                        xt = xpool.tile([C, 2048], fp16, tag="x")
                        nc.gpsimd.dma_start(
                            out=xt, in_=x_slice[:, tf * 2048 : (tf + 1) * 2048]
                        )
                        th = psA.tile([128, F], f32, tag="pgbank", name="psThfill")
                        for c in range(4):
                            nc.tensor.matmul(
                                th[32 * c : 32 * c + 32, :],
                                w_th_sb,
                                xt[:, c * 512 : (c + 1) * 512],
                                start=True, stop=True, tile_position=(0, 32 * c),
                            )
                        th4 = th4pool.tile([128, F], bf16, tag="th4")
                        nc.scalar.copy(th4, th)
                        for c in range(4):
                            nc.sync.dma_start(
                                out=theta_sb[
                                    0:8, tf * 2048 + c * 512 : tf * 2048 + (c + 1) * 512
                                ],
                                in_=th4[32 * c : 32 * c + 8, :],
                            )
                    return go

                for tf in range(4):
                    yield fill(tf)

                def replicate():
                    for off in (32, 64):
                        nc.sync.dma_start(
                            out=theta_sb[off : off + 8, :], in_=theta_sb[0:8, :]
                        )
                yield replicate

            a_steps = list(theta_steps())
            for q in range(4):
                a_steps.extend(phase_a_quarter_steps(q))
            a_idx = 0

            def drain_a(k):
                nonlocal a_idx
                for _ in range(k):
                    if a_idx < len(a_steps):
                        a_steps[a_idx]()
                        a_idx += 1

            # Emit theta + quarter 0 up front; quarters 1-3 interleave below.
            drain_a(5 + 11)

            # ---------------- Phase C: flash attention -----------------------
            if True:
                def make_state(t):
                    return {
                        "t": t,
                        "n0": t * F,
                        "expS": bigpool.tile([128, MC, F], bf16, tag="big", name="expS"),
                        "psO": psO_p.tile([128, F], f32, name="psO", tag="psO"),
                        "ready": 0,
                        "odone": 0,
                        "ne": 0,
                        "no": 0,
                    }

                def emit_group(st, gi):
                    mc0, mc1 = GROUPS[gi]
                    cnt = mc1 - mc0
                    sps = psS.tile([128, 3 * F], f32, tag="psS", name="sps")
                    for i, mc in enumerate(range(mc0, mc1)):
                        nc.tensor.matmul(
                            sps[:, i * F : (i + 1) * F],
                            phi_sb[32 * i : 32 * i + 8, mc * 128 : (mc + 1) * 128],
                            theta_sb[32 * i : 32 * i + 8, st["n0"] : st["n0"] + F],
                            start=True,
                            stop=True,
                            tile_position=(32 * i, 0),
                        )
                    if gi in _dve_groups(st["t"]):
                        nc.vector.tensor_scalar(
                            st["expS"][:, mc0:mc1, :].bitcast(i16),
                            sps[:, 0 : cnt * F],
                            A16,
                            B16,
                            Mult,
                            Add,
                        )
                    else:
                        nc.scalar.activation(
                            out=st["expS"][:, mc0:mc1, :],
                            in_=sps[:, 0 : cnt * F],
                            func=Exp,
                        )
                    st["ready"] = mc1

                def emit_o(st):
                    mc = st["odone"]
                    par = mc % 2
                    first = (st["ne"] if par == 0 else st["no"]) == 0
                    last = (st["ne"] if par == 0 else st["no"]) == MC // 2 - 1
                    nc.tensor.matmul(
                        st["psO"][0:64, :] if par == 0 else st["psO"][64:128, :],
                        gt[:, mc, :],
                        st["expS"][:, mc, :],
                        start=first,
                        stop=last,
                        tile_position=(0, 0) if par == 0 else (0, 64),
                    )
                    if par == 0:
                        st["ne"] += 1
                    else:
                        st["no"] += 1
                    st["odone"] += 1

                def tail_part1(st):
                    # merge even/odd sums; separate den TT (the custom-DVE
                    # reciprocal requires a base-0 in-place operand)
                    psO = st["psO"]
                    tmp = smallpool.tile([33, F], f32, tag="ob")
                    nc.vector.tensor_copy(tmp, psO[64:97, :])
                    o32 = smallpool.tile([32, F], f32r, tag="o33")
                    nc.vector.tensor_tensor(o32, psO[0:32, :], tmp[0:32, :], Add)
                    den = smallpool.tile([1, F], f32, tag="den")
                    nc.vector.tensor_tensor(den, psO[32:33, :], tmp[32:33, :], Add)
                    st["o32"] = o32
                    st["den"] = den
                    if DBG and st["t"] == 0:
                        d1 = outpool.tile([33, F], f32, name="d1", tag="dbg1", bufs=1)
                        nc.vector.tensor_copy(d1[0:32], o32[:].bitcast(f32))
                        nc.vector.tensor_copy(d1[32:33], den)
                        nc.sync.dma_start(out=dbg_o33[:], in_=d1)
                        for pc in range(4):
                            d2 = outpool.tile([128, MC // 4, F], f32, name="d2", tag="dbg2", bufs=1)
                            nc.vector.tensor_copy(d2, st["expS"][:, pc * 8 : (pc + 1) * 8, :])
                            nc.sync.dma_start(out=dbg_exp[:, pc * 8 : (pc + 1) * 8, :], in_=d2)
                    xres = xpool.tile([C, F], f32, tag="xres", bufs=2)
                    nc.gpsimd.dma_start(out=xres, in_=x_slice[:, st["n0"] : st["n0"] + F])
                    st["xres"] = xres

                def tail_part2(st):
                    den = st["den"]
                    nc.vector.reciprocal_approx_fast(out=den, in_=den)
                    rb = smallpool.tile([64, F], f32, tag="rb")
                    nc.gpsimd.partition_broadcast(rb, den)
                    st["rb"] = rb
                    if DBG and st["t"] == 0:
                        nc.sync.dma_start(out=dbg_den[:], in_=den)
                    # project unnormalized o; (w_o@o)/den == w_o@(o/den)
                    nc.tensor.matmul(
                        st["psO"][0:64, :],
                        w_oT_sb,
                        st["o32"],
                        start=True,
                        stop=True,
                    )

                def tail_part3(st):
                    pn = smallpool.tile([64, F], f32, tag="pn")
                    nc.vector.tensor_mul(pn, st["psO"][0:64, :], st["rb"])
                    ot = outpool.tile([C, F], f32)
                    nc.vector.tensor_add(ot, pn, st["xres"])
                    nc.gpsimd.dma_start(out=out_d[:, st["n0"] : st["n0"] + F], in_=ot)

                prev = None
                for t in range(NT):
                    st = make_state(t)
                    for gi in range(NG):
                        drain_a(4 if t < 2 else 2)
                        emit_group(st, gi)
                        if prev is not None:
                            # tails early: the single psO bank must be fully
                            # read before this tile's o-matmuls start
                            if gi == 0:
                                tail_part1(prev)
                            elif gi == 1:
                                tail_part2(prev)
                            elif gi == 2:
                                tail_part3(prev)
                                prev = None
                        else:
                            # lag o-matmuls ~2 groups behind exp (sems are
                            # pre-satisfied, so even/odd couples co-issue and
                            # col-tile on the PE); strictly paired emission
                            while st["odone"] + 1 < st["ready"] - 6:
                                emit_o(st)
                                emit_o(st)
                    while st["odone"] < MC:
                        emit_o(st)
                    prev = st
                tail_part1(prev)
                tail_part2(prev)
                tail_part3(prev)
                psA.release()
                psO_p.release()
                psS.release()
                if DBG:
                    dp = outpool.tile([72, M], f32, name="dp", tag="dbg3", bufs=1)
                    nc.vector.tensor_copy(dp, phi_sb[:])
                    nc.sync.dma_start(out=dbg_phi[:], in_=dp)
                    for hw in range(4):
                        dt_ = outpool.tile([96, NS // 4], f32, name="dt_", tag="dbg4", bufs=1)
                        nc.vector.tensor_copy(dt_, theta_sb[:, hw * 2048 : (hw + 1) * 2048])
                        nc.sync.dma_start(out=dbg_th[:, hw * 2048 : (hw + 1) * 2048], in_=dt_)
                    dg = outpool.tile([128, MC * 64], f32, name="dg", tag="dbg5", bufs=1)
                    nc.vector.tensor_copy(dg, gtv[:])
                    nc.sync.dma_start(out=dbg_gt[:], in_=dg)



    nc.finalize()
    return nc


def _maybe_trace_setup():
    """Optional NTFF profiling (test harness only, via NLATTN_TRACE=1)."""
    if not os.environ.get("NLATTN_TRACE"):
        return False
    import types

    try:
        from antenv.axon_hooks import get_axon_ntff_profile_hook  # noqa: F401
    except ImportError:
        import antenv

        mod = types.ModuleType("antenv.axon_hooks")
        mod._hook = None

        def set_axon_ntff_profile_hook(h):
            mod._hook = h

        def get_axon_ntff_profile_hook():
            return mod._hook

        mod.set_axon_ntff_profile_hook = set_axon_ntff_profile_hook
        mod.get_axon_ntff_profile_hook = get_axon_ntff_profile_hook
        sys.modules["antenv.axon_hooks"] = mod
        antenv.axon_hooks = mod
        from trn_agent_boot.trn_boot import _ntff_profile_via_ctypes

        mod._hook = _ntff_profile_via_ctypes("/opt/axon/libaxon_pjrt.so")
    import concourse.bass_utils as bu

    bu.upload_artifacts = lambda tmpdir: "local://" + str(tmpdir)
    return True


_LAST_RESULT = {}


def kernel(x, w_theta, w_phi, w_g, w_o, gamma):
    from concourse.bass_utils import run_bass_kernel_spmd

    trace = _maybe_trace_setup()

    B = np.asarray(x).shape[0]
    xf = np.ascontiguousarray(np.asarray(x).reshape(B, C, N), dtype=np.float32)
    w_pg_h = np.ascontiguousarray(
        np.concatenate(
            [np.asarray(w_g), np.asarray(w_phi), np.zeros((24, C), np.float32)],
            axis=0,
        ).T,
        dtype=np.float32,
    )
    w_th_h = np.ascontiguousarray(
        np.concatenate([np.asarray(w_theta), np.zeros((24, C), np.float32)], axis=0).T,
        dtype=np.float32,
    )
    w_oT_h = np.ascontiguousarray(np.asarray(w_o).T, dtype=np.float32)
    gamma_h = np.asarray(gamma, dtype=np.float32).reshape(1, 1)

    nc = _build_program()

    in_maps = []
    for core in range(8):
        b, s = core // 4, core % 4
        in_maps.append(
            {
                "x_full": xf[b],
                "x_slice": np.ascontiguousarray(xf[b][:, s * NS : (s + 1) * NS]),
                "w_pg": w_pg_h,
                "w_th": w_th_h,
                "w_oT": w_oT_h,
                "gamma": gamma_h,
            }
        )

    res = run_bass_kernel_spmd(nc, in_maps, core_ids=list(range(8)), trace=trace)
    _LAST_RESULT["exec_time_ns"] = res.exec_time_ns
    _LAST_RESULT["trace"] = res.instructions_and_trace

    out = np.empty((B, C, N), dtype=np.float32)
    for core in range(8):
        b, s = core // 4, core % 4
        out[b][:, s * NS : (s + 1) * NS] = res.results[core]["out"]
    D = H = W = 32
    return out.reshape(B, C, D, H, W)
